# revision 58
# baseline (speedup 1.0000x reference)
"""Conv2d 3x3 (stride 1, pad 1) Bass kernel for TRN2, 8-core SPMD.

Problem: x [32, 64, 56, 56] f32, filters [128, 64, 3, 3] f32
         -> out [32, 128, 56, 56] f32.

Sharding: data-parallel over batch, 4 images per core.

Per-core layout:
  - Host pads each image to [64c, 58, 58] (zero border), casts to bf16,
    and places channels of images {0,1} in SBUF partitions 0-63 and
    channels of images {2,3} in partitions 64-127.  One [128, 6728]
    bf16 tensor.
  - Conv = 9 shifted K=64 matmuls (taps) accumulated in PSUM.  The two
    partition halves run as concurrent row-tiled matmuls (tile_position
    rows 0/64) producing two independent output tiles (different
    images) per round.
  - bf16 operands: 1 cycle/row on the PE, half the HBM traffic of
    fp32.  PSUM accumulation stays fp32; output is stored bf16 and
    upcast to fp32 on the host (~0.3% rel err, gate is 2e-2).
  - DMA: one packed input tensor in DELIVERY order (all weights +
    strip-0 rows 0-10 as a single one-packet-per-row transfer, then
    row chunks alternating strips), all on the sync HWDGE ring —
    single-ring FIFO is deterministic, cross-ring SDMA round-robin is
    not.  Blocks process strips interleaved (s0-rb0, s1-rb0, s0-rb1,
    ...) so every chunk has ~3.4us of slack over its consumer: the
    matmul stream never stalls on input.  Outputs split across scalar
    (image pair a) and sync (pair b) rings.
  - A 10-matmul PE warm-up chain bridges the ~3.4us HAM clock-gate
    window between engine start (~7us) and first data (~10.5us) so the
    stream runs at 2.4GHz from the first real matmul; any PE idle gap
    ~1us there re-throttles the clock to 1.2GHz for several
    microseconds (measured: a 43us run from exactly that).
"""

import sys

sys.path.insert(0, "/opt/trn_rl_repo")

import numpy as np

B, C, H, W = 32, 64, 56, 56
OC = 128
KH = KW = 3
NCORES = 8
BPC = B // NCORES          # images per core (4)
HP, WP = H + 2, W + 2      # padded 58x58
IMG = HP * WP              # 3364 padded image size per channel
STRIP = 2                  # images per partition-strip
L = STRIP * IMG            # free-dim length of the x tensor (6728)
RB = 8                     # output rows per tile
NT = RB * W                # matmul free size (448)
NRB = H // RB              # row blocks per image (7)
OUT_IMG = H * W            # 3136

_cache = {}


def _build():
    import concourse.mybir as mybir
    import concourse.tile as tile
    from concourse import bacc

    nc = bacc.Bacc("TRN2", target_bir_lowering=False, debug=False,
                   num_devices=NCORES)
    # One packed input tensor, laid out in DELIVERY order: weights
    # first, then x row-chunks interleaved across the two strips to
    # match the strip-interleaved block processing order below.
    WCOLS = KH * KW * OC
    x_ext = nc.declare_dram_parameter("x2", [2 * C, WCOLS + L],
                                      mybir.dt.bfloat16, isOutput=False)
    y_ext = nc.declare_dram_parameter("y", [BPC, OC, OUT_IMG],
                                      mybir.dt.bfloat16, isOutput=True)

    with tile.TileContext(nc) as tc:
        with (
            tc.tile_pool(name="xp", bufs=1) as xp,
            tc.tile_pool(name="wp", bufs=1) as wp,
            tc.tile_pool(name="ps", bufs=4, space="PSUM") as ps,
            tc.tile_pool(name="op", bufs=8) as op,
        ):
            xw_t = xp.tile([2 * C, WCOLS + L], mybir.dt.bfloat16)
            w_t = xw_t[:, 0:WCOLS]
            x_t = xw_t[:, WCOLS:]
            # All input transfers on the sync ring in delivery order
            # (single-ring FIFO is deterministic).  First transfer =
            # all weights + strip-0 rows 0-10; then row chunks
            # alternate strips to match the block order below, giving
            # each chunk ~3.4us of slack before its consumer.
            row_bounds = (0, 10 * WP, 26 * WP, 42 * WP, IMG)
            segs = [(0, WCOLS + 10 * WP, 0)]      # (src_lo, src_hi, dst_lo)
            src = WCOLS + 10 * WP
            segs.append((src, src + 10 * WP, WCOLS + IMG))
            src += 10 * WP
            for bi in range(1, len(row_bounds) - 1):
                lo, hi = row_bounds[bi], row_bounds[bi + 1]
                for q in range(STRIP):
                    segs.append((src, src + (hi - lo), WCOLS + q * IMG + lo))
                    src += hi - lo
            for (slo, shi, dlo) in segs:
                nc.sync.dma_start(xw_t[:, dlo:dlo + (shi - slo)],
                                  x_ext.ap()[:, slo:shi])
            x4 = x_t.rearrange("p (i r w) -> p i r w", i=STRIP, w=WP)

            # PE warm-up on an uninitialized scratch tile (no deps at
            # all): keeps the HAM activity window hot from ~7.3us until
            # the first data lands (~10.0us) so the clock gate releases
            # early.  The results are garbage into a never-read PSUM
            # bank; only the activity matters.
            wsrc = wp.tile([2 * C, 512], mybir.dt.bfloat16, tag="warmsrc")
            # one-column memset just to mark the tile written/allocated;
            # the rest is read as garbage (results are never consumed)
            nc.gpsimd.memset(wsrc[:, 0:1], 0.0)
            warm = ps.tile([OC, NT], mybir.dt.float32, tag="pa")
            for _ in range(10):
                nc.tensor.matmul(warm[:], wsrc[:, 0:OC], wsrc[:, 0:NT],
                                 start=True, stop=True,
                                 skip_group_check=True)

            def do_block(q, h0, rows):
                """One PSUM accumulation group: `rows` output rows of
                image pair (q, q+2) starting at output row h0."""
                n = rows * W
                pa = ps.tile([OC, NT], mybir.dt.float32, tag="pa")
                pb = ps.tile([OC, NT], mybir.dt.float32, tag="pb")
                for tap in range(KH * KW):
                    kh, kw = divmod(tap, KW)
                    hh = h0 + kh
                    rhs_a = x4[0:C, q, hh:hh + rows, kw:kw + W]
                    rhs_b = x4[C:2 * C, q, hh:hh + rows, kw:kw + W]
                    wsl = slice(tap * OC, (tap + 1) * OC)
                    nc.tensor.matmul(
                        pa[:, 0:n], w_t[0:C, wsl], rhs_a,
                        start=(tap == 0), stop=(tap == KH * KW - 1))
                    nc.tensor.matmul(
                        pb[:, 0:n], w_t[C:2 * C, wsl], rhs_b,
                        start=(tap == 0), stop=(tap == KH * KW - 1))
                oa = op.tile([OC, NT], mybir.dt.bfloat16, tag="oa")
                ob = op.tile([OC, NT], mybir.dt.bfloat16, tag="ob")
                # evacuate the two PSUM tiles on different engines so
                # the copies (and the final tail) run in parallel
                nc.vector.tensor_copy(oa[:, 0:n], pa[:, 0:n])
                nc.scalar.activation(ob[:, 0:n], pb[:, 0:n],
                                     mybir.ActivationFunctionType.Copy)
                sl = slice(h0 * W, h0 * W + n)
                # outputs split across both HWDGE rings: the scalar
                # ring is otherwise idle; the sync-ring outputs queue
                # behind the input bulk but nothing waits on them
                # until the epilogue
                nc.scalar.dma_start(y_ext.ap()[q, :, sl], oa[:, 0:n])
                nc.sync.dma_start(y_ext.ap()[q + STRIP, :, sl], ob[:, 0:n])

            # blocks interleave strips (s0-rb0, s1-rb0, s0-rb1, ...) so
            # each input chunk has two block-periods (~3.4us) of slack
            for r in range(NRB):            # 8-row block
                for q in range(STRIP):      # image within strip
                    if q == STRIP - 1 and r == NRB - 1:
                        # split the final block so the tail chain
                        # (copy -> desc-gen -> transfer -> receipt) runs
                        # on a half-size tile
                        do_block(q, r * RB, RB // 2)
                        do_block(q, r * RB + RB // 2, RB // 2)
                    else:
                        do_block(q, r * RB, RB)

    nc.compile()
    return nc


def _prep_inputs(x, filters):
    """Host-side reshape/pad/cast: returns per-core in_maps."""
    import ml_dtypes

    bf16 = ml_dtypes.bfloat16
    xpad = np.zeros((B, C, HP, WP), dtype=np.float32)
    xpad[:, :, 1:1 + H, 1:1 + W] = x
    xpad = xpad.astype(bf16)
    # [B, C, HP, WP] -> per core [2C, L]
    wt = np.empty((2 * C, KH * KW * OC), dtype=np.float32)
    for tap in range(KH * KW):
        kh, kw = divmod(tap, KW)
        wtap = filters[:, :, kh, kw].T.astype(np.float32)  # [C, OC]
        wt[0:C, tap * OC:(tap + 1) * OC] = wtap
        wt[C:2 * C, tap * OC:(tap + 1) * OC] = wtap
    wt = wt.astype(bf16)
    in_maps = []
    for c in range(NCORES):
        xc = xpad[c * BPC:(c + 1) * BPC]                   # [4, C, HP, WP]
        lower = xc[0:2].transpose(1, 0, 2, 3).reshape(C, L)
        upper = xc[2:4].transpose(1, 0, 2, 3).reshape(C, L)
        xs = np.concatenate([lower, upper], axis=0)        # [2C, L]
        s0, s1 = xs[:, 0:IMG], xs[:, IMG:]
        # pack in delivery order: weights, strip-0 rows 0-10, strip-1
        # rows 0-10, then row chunks 10-26/26-42/42-58 alternating
        # strips (matches the kernel's DMA seg list)
        rb_b = (0, 10 * WP, 26 * WP, 42 * WP, IMG)
        parts = [wt, s0[:, 0:10 * WP], s1[:, 0:10 * WP]]
        for bi in range(1, len(rb_b) - 1):
            lo, hi = rb_b[bi], rb_b[bi + 1]
            parts.append(s0[:, lo:hi])
            parts.append(s1[:, lo:hi])
        x2 = np.ascontiguousarray(np.concatenate(parts, axis=1))
        in_maps.append({"x2": x2})
    return in_maps


def kernel(x, filters):
    from concourse.bass_utils import run_bass_kernel_spmd

    x = np.asarray(x, dtype=np.float32)
    filters = np.asarray(filters, dtype=np.float32)
    if "nc" not in _cache:
        _cache["nc"] = _build()
    nc = _cache["nc"]
    in_maps = _prep_inputs(x, filters)
    res = run_bass_kernel_spmd(nc, in_maps, core_ids=list(range(NCORES)))
    out = np.empty((B, OC, H, W), dtype=np.float32)
    for c in range(NCORES):
        y = res.results[c]["y"]                            # [4, OC, 3136] bf16
        out[c * BPC:(c + 1) * BPC] = np.asarray(y, dtype=np.float32).reshape(
            BPC, OC, H, W)
    return out


if __name__ == "__main__":
    rng = np.random.default_rng(0)
    x = rng.standard_normal((B, C, H, W), dtype=np.float32)
    f = rng.standard_normal((OC, C, KH, KW), dtype=np.float32)
    out = kernel(x, f)
    print("out", out.shape, out.dtype, float(np.abs(out).mean()))
